# revision 29
# baseline (speedup 1.0000x reference)
"""Trainium2 Bass kernel for a BasicTransformerBlock (self-attn + cross-attn + GEGLU FFN).

Sharding: pure data-parallel over (batch, query-rows). 8 cores = 2 batches x 4
query-slices of 1024 rows. Only the self-attention K/V path needs all 4096
tokens of a batch element, and K/V are recomputed per core from the (shared)
input x, so there are no collectives at all.

Key engine-balance tricks vs the straightforward lowering:
  * LN1/LN2 are folded into the projections:  LN(x)@W = r_t*(x@W) - (m_t*r_t)*colsum(W)
    so the QKV matmuls start on RAW x immediately (no serial LN prologue) and
    the fix-up is two cheap 2-byte DVE ops per tile (colsums shipped from host).
  * LN1 stats are computed with PE ones-matmuls (sum x, sum x^2) interleaved
    with the QKV projections.
  * softmax exp is split across engines: even head of each pair on the Scalar
    (ACT) engine (exact exp), odd head on the Vector engine via a Schraudolph
    bf16 bit-trick: int16(A*s + B) reinterpreted as bf16 == 2^(frac) piecewise
    approx of exp. Its uniform component cancels exactly in the softmax
    normalization; the +-3% sawtooth averages out over 4096 keys.
  * residual stream kept in bf16; attention-output normalization multiplies on
    GpSimd; V-path psum->sbuf casts on ACT (per-partition activation scale).
"""

import numpy as np
import ml_dtypes
from contextlib import ExitStack

import concourse.bass as bass
import concourse.tile as tile
from concourse import bacc, mybir
from concourse.bass_utils import run_bass_kernel_spmd

AF = mybir.ActivationFunctionType
ALU = mybir.AluOpType
BF16 = mybir.dt.bfloat16
F32 = mybir.dt.float32
I16 = mybir.dt.int16

DIM = 512
H = 8
D = 64
B = 2
S = 4096
TCTX = 77
NCORES = 8
Q = 1024          # query rows per core
P = 128
CC = DIM // P     # contraction chunks of 128
EPS = 1e-5
SCALE = D ** -0.5
LOG2E = 1.4426950408889634
# Schraudolph constants for bf16-bit exp: bits = SCHA*score + SCHB (int16),
# bitcast to bf16 ~= exp(score*SCALE). +0.5 compensates truncation.
SCHA = SCALE * LOG2E * 128.0
SCHB = 127.0 * 128.0 - 7.4 + 0.5

_CACHE = {}


def _bcast_dram_ap(ap, nparts):
    """DMA source AP that broadcasts a DRAM row across nparts partitions."""
    return bass.AP(tensor=ap.tensor, offset=ap.offset, ap=[[0, nparts]] + list(ap.ap))


def _body(ctx, tc, a):
    nc = tc.nc
    persist = ctx.enter_context(tc.tile_pool(name="persist", bufs=1))

    def open_pool(name, side="left"):
        cm = tc.tile_pool(name=name, bufs=1, side=side)
        pool = cm.__enter__()
        return cm, pool

    eps_t = persist.tile([P, 1], F32, tag="eps")
    nc.vector.memset(eps_t[:], EPS)
    ones_b = persist.tile([P, 1], BF16, tag="ones_b")
    nc.vector.memset(ones_b[:], 1.0)

    # ---- LN fold-in scalars from host -----------------------------------
    ncs_q1 = persist.tile([P, CC], F32, tag="ncs_q1")
    ncs_q2 = persist.tile([P, CC], F32, tag="ncs_q2")
    nc.sync.dma_start(out=ncs_q1, in_=a["ncs_q1"])
    nc.sync.dma_start(out=ncs_q2, in_=a["ncs_q2"])
    cs_k1 = persist.tile([P, CC], F32, tag="cs_k1")      # +colsum(wk1)
    nc.sync.dma_start(out=cs_k1, in_=a["cs_k1"])
    cvc = persist.tile([P, H], F32, tag="cvc")   # colsum(wv1): row hh*64+d, col h
    nc.sync.dma_start(out=cvc, in_=a["cv_col"])

    # ---------------- load xT + weights; QKV on RAW x --------------------
    cm_x, pool_x = open_pool("pool_x")          # XT + stats rows: die after QKV
    XT = []
    for c in range(CC):
        t = pool_x.tile([P, S], BF16, tag=f"XT{c}")
        XT.append(t)
    # DMA xT in token-chunk order so stats/projections pipeline per chunk
    NTC = 8
    TCW = S // NTC  # 512
    for tch in range(NTC):
        for c in range(CC):
            nc.sync.dma_start(out=XT[c][:, tch * TCW:(tch + 1) * TCW],
                              in_=a["xT"][c * P:(c + 1) * P, tch * TCW:(tch + 1) * TCW])

    cm_at, pool_at = open_pool("pool_at", side="right")  # KT, QT, VP, O1T
    KTP = [pool_at.tile([P, S], BF16, tag=f"KT{p}", name=f"KT{p}") for p in range(4)]
    QTP = [pool_at.tile([P, Q], BF16, tag=f"QT{p}", name=f"QT{p}") for p in range(4)]
    VP = pool_at.tile([P, 32, H, D + 2], BF16, tag="VP")
    nc.vector.memset(VP[:, :, :, D:D + 1], 1.0)
    XRES = pool_at.tile([P, CC, Q], BF16, tag="XRES")    # residual x (own queries)

    # stats rows (bf16 - same precision as a bf16 rstd elsewhere) for LN1
    r_row = pool_x.tile([1, S], BF16, tag="r_row")
    s_rowb = pool_x.tile([1, S], BF16, tag="s_rowb")
    mn_row = pool_x.tile([1, S], BF16, tag="mn_row")     # -m
    # broadcast tiles (feature-on-partition consumers)
    r_b = pool_at.tile([P, Q], BF16, tag="r_b")          # Q-fix only
    s_b = pool_at.tile([P, Q], BF16, tag="s_b")          # Q-fix only
    mn_b = pool_at.tile([P, S], BF16, tag="mn_b")        # K-fix
    # token-on-partition stats (partition p of col tb = token tb*128+p)
    mv = pool_x.tile([P, 32, 2], F32, tag="mv")
    rcol = pool_at.tile([P, 32], F32, tag="rcol")
    sncol = pool_at.tile([P, 32], F32, tag="sncol")      # -m*r
    rscl = pool_at.tile([P, 32], F32, tag="rscl")        # r * softmax scale
    rsca = pool_at.tile([P, 32], F32, tag="rsca")        # r * schraudolph A

    def load_w(pool, name, rows=DIM, cols=DIM):
        ts = []
        for c in range(rows // P):
            t = pool.tile([P, cols], BF16, tag=f"{name}{c}")
            nc.sync.dma_start(out=t, in_=a[name][c * P:(c + 1) * P, :])
            ts.append(t)
        return ts

    # cross-attn K2/V2 (context only) - emitted early, runs whenever PE has slack
    cm_w2, wp2 = open_pool("pool_w2")
    WK2 = load_w(wp2, "wk2")
    WV2 = load_w(wp2, "wv2")
    CTX = []
    for c in range(CC):
        t = wp2.tile([P, TCTX], BF16, tag=f"CTX{c}")
        nc.sync.dma_start(out=t, in_=a["ctxT"][c * P:(c + 1) * P, :])
        CTX.append(t)
    K2TP = [persist.tile([P, TCTX], BF16, tag=f"K2T{p}", name=f"K2T{p}") for p in range(4)]
    VP2 = persist.tile([TCTX, H, D + 1], BF16, tag="VP2")
    nc.vector.memset(VP2[:, :, D:D + 1], 1.0)

    cm_w1, wp1 = open_pool("pool_w1")
    WQ = load_w(wp1, "wq1")
    WK = load_w(wp1, "wk1")
    WV = load_w(wp1, "wv1")

    with tc.tile_pool(name="qkvps", bufs=6, space="PSUM") as pp, \
         tc.tile_pool(name="kv2ps", bufs=1, space="PSUM") as kpp, \
         tc.tile_pool(name="xnp", bufs=4) as xnp, \
         tc.tile_pool(name="lnsb", bufs=2) as lsb:
        # K2/V2 first (tiny, no deps beyond ctx dma)
        for p4 in range(4):
            psk = kpp.tile([P, TCTX], F32, tag="psk")
            for c in range(CC):
                nc.tensor.matmul(psk[:], lhsT=WK2[c][:, p4 * P:(p4 + 1) * P], rhs=CTX[c][:],
                                 start=(c == 0), stop=(c == CC - 1))
            nc.vector.tensor_copy(out=K2TP[p4][:], in_=psk[:])
        psv = kpp.tile([TCTX, 512], F32, tag="psv")
        for c in range(CC):
            nc.tensor.matmul(psv[:], lhsT=CTX[c][:], rhs=WV2[c][:],
                             start=(c == 0), stop=(c == CC - 1))
        nc.vector.tensor_copy(out=VP2[:, :, 0:D],
                              in_=psv[:].rearrange("p (h d) -> p h d", h=H))

        # LN1 stats first: xnat DMA + bn_stats for all 32 token blocks, so the
        # fix-up scalars are ready while projections are still streaming.
        for tb in range(32):
            xn = xnp.tile([P, DIM], BF16, tag="xn")
            nc.sync.dma_start(out=xn, in_=a["xnat"][tb * P:(tb + 1) * P, :])
            st6 = lsb.tile([P, 6], F32, tag="st6")
            nc.vector.bn_stats(out=st6, in_=xn)
            nc.vector.bn_aggr(out=mv[:, tb, :], in_=st6)
        # rsqrt on ACT in token-partition layout, then rows via a
        # scatter-to-DRAM round trip (t = tb*128 + p ordering)
        lnv32 = lsb.tile([P, 32], F32, tag="lnv32")
        nc.scalar.activation(out=lnv32[:], in_=mv[:, :, 1], func=AF.Ln, bias=eps_t[:])
        nc.scalar.activation(out=rcol[:], in_=lnv32[:], func=AF.Exp, scale=-0.5)
        nc.vector.scalar_tensor_tensor(out=sncol[:], in0=mv[:, :, 0], scalar=-1.0,
                                       in1=rcol[:], op0=ALU.mult, op1=ALU.mult)
        nc.vector.tensor_scalar(out=rscl[:], in0=rcol[:], scalar1=SCALE, scalar2=None,
                                op0=ALU.mult)
        nc.vector.tensor_scalar(out=rsca[:], in0=rcol[:], scalar1=SCHA, scalar2=None,
                                op0=ALU.mult)
        # VP column D+1 = sn_j = -m_j*r_j (accumulates the V s-term via attnV)
        for tb in range(32):
            snsrc = bass.AP(tensor=sncol[:].tensor, offset=sncol[:].offset + tb,
                            ap=[[32, P], [0, H]])
            nc.vector.tensor_copy(out=VP[:, tb, :, D + 1], in_=snsrc)
        rch = lsb.tile([P, 32], BF16, tag="rch")
        sch = lsb.tile([P, 32], BF16, tag="sch")
        mch = lsb.tile([P, 32], BF16, tag="mch")
        nc.vector.tensor_copy(out=rch[:], in_=rcol[:])
        nc.vector.tensor_scalar(out=sch[:], in0=sncol[:], scalar1=-1.0, scalar2=None,
                                op0=ALU.mult)
        nc.vector.tensor_scalar(out=mch[:], in0=mv[:, :, 0], scalar1=-1.0, scalar2=None,
                                op0=ALU.mult)
        r_dram = pool_x.tile([1, S], BF16, tag="r_dram", space="DRAM")
        s_dram = pool_x.tile([1, S], BF16, tag="s_dram", space="DRAM")
        m_dram = pool_x.tile([1, S], BF16, tag="m_dram", space="DRAM")
        for srct, dr, rowt in ((rch, r_dram, r_row), (sch, s_dram, s_rowb),
                               (mch, m_dram, mn_row)):
            dst = bass.AP(tensor=dr[:].tensor, offset=dr[:].offset,
                          ap=[[1, P], [P, 32]])
            nc.sync.dma_start(out=dst, in_=srct[:])
            nc.sync.dma_start(out=rowt, in_=dr[:])
        nc.gpsimd.partition_broadcast(out_ap=mn_b[:], in_ap=mn_row[:], channels=P)
        nc.gpsimd.partition_broadcast(out_ap=r_b[:], in_ap=r_row[:, 0:Q], channels=P)
        nc.gpsimd.partition_broadcast(out_ap=s_b[:], in_ap=s_rowb[:, 0:Q], channels=P)

        # residual copy of own-query x columns (bf16)
        for c in range(CC):
            nc.vector.tensor_copy(out=XRES[:, c, :], in_=XT[c][:, 0:Q])

        # raw projections (casts on ACT) with inline LN1 fix-ups trailing
        for tch in range(NTC):
            ts = slice(tch * TCW, (tch + 1) * TCW)
            for p4 in range(4):
                ps = pp.tile([P, TCW], F32, tag="ps", name=f"psk_{tch}_{p4}")
                for c in range(CC):
                    nc.tensor.matmul(ps[:], lhsT=WK[c][:, p4 * P:(p4 + 1) * P],
                                     rhs=XT[c][:, ts],
                                     start=(c == 0), stop=(c == CC - 1))
                nc.scalar.copy(out=KTP[p4][:, ts], in_=ps[:])
                nc.vector.scalar_tensor_tensor(
                    out=KTP[p4][:, ts], in0=mn_b[:, ts], scalar=cs_k1[:, p4:p4 + 1],
                    in1=KTP[p4][:, ts], op0=ALU.mult, op1=ALU.add)
            if tch < 2:
                for p4 in range(4):
                    ps = pp.tile([P, TCW], F32, tag="ps", name=f"psq_{tch}_{p4}")
                    for c in range(CC):
                        nc.tensor.matmul(ps[:], lhsT=WQ[c][:, p4 * P:(p4 + 1) * P],
                                         rhs=XT[c][:, ts],
                                         start=(c == 0), stop=(c == CC - 1))
                    nc.scalar.copy(out=QTP[p4][:, ts], in_=ps[:])
                    nc.vector.scalar_tensor_tensor(
                        out=QTP[p4][:, ts], in0=QTP[p4][:, ts], scalar=1.0,
                        in1=r_b[:, ts], op0=ALU.mult, op1=ALU.mult)
                    nc.vector.scalar_tensor_tensor(
                        out=QTP[p4][:, ts], in0=s_b[:, ts], scalar=ncs_q1[:, p4:p4 + 1],
                        in1=QTP[p4][:, ts], op0=ALU.mult, op1=ALU.add)
            for tb in range(4 * tch, 4 * tch + 4):
                ps = pp.tile([P, 512], F32, tag="ps", name=f"psv_{tb}")
                for c in range(CC):
                    nc.tensor.matmul(ps[:], lhsT=XT[c][:, tb * P:(tb + 1) * P], rhs=WV[c][:],
                                     start=(c == 0), stop=(c == CC - 1))
                nc.scalar.activation(out=VP[:, tb, :, 0:D],
                                     in_=ps[:].rearrange("p (h d) -> p h d", h=H),
                                     func=AF.Copy, scale=rcol[:, tb:tb + 1])

    cm_w1.__exit__(None, None, None)
    cm_w2.__exit__(None, None, None)
    cm_x.__exit__(None, None, None)

    # preload all later-phase weights now; DMAs overlap the attention phase
    cm_wl, wpl = open_pool("pool_wlate")
    WO1 = load_w(wpl, "wo1")
    WQ2 = load_w(wpl, "wq2")
    WO2 = load_w(wpl, "wo2")
    GW = load_w(wpl, "gw", cols=8 * DIM)
    OW = load_w(wpl, "ow", rows=4 * DIM)

    # ---------------- Phase D: self-attention ---------------------------------
    O1T = pool_at.tile([P, CC, Q], BF16, tag="O1T")
    with tc.tile_pool(name="spool", bufs=2, space="PSUM") as spool, \
         tc.tile_pool(name="opool", bufs=4, space="PSUM") as opool, \
         tc.tile_pool(name="ppool", bufs=4) as ppool, \
         tc.tile_pool(name="npool", bufs=2) as npool:
        for qb in range(2):
            for p4 in range(4):
                oo = []
                for hh in range(2):
                    o66 = opool.tile([D + 2, 512], F32, tag="o65", name=f"o65_{qb}_{p4}_{hh}")
                    oo.append(o66)
                # staggered software pipeline: attnV-h0 lags scores by 2,
                # attnV-h1 by 3 (its exp comes off the slower DVE path)
                pend0 = []
                pend1 = []

                def attn_v1(pkb, pp1):
                    nc.tensor.matmul(oo[1][:], lhsT=VP[:, pkb, 2 * p4 + 1, :], rhs=pp1,
                                     start=(pkb == 0), stop=(pkb == 31))

                for kb in range(32):
                    # both heads' scores into one 2-bank psum region
                    s2t = spool.tile([P, 2, 512], F32, tag="S")
                    for hh in range(2):
                        nc.tensor.matmul(
                            s2t[:, hh, :],
                            lhsT=KTP[p4][hh * D:(hh + 1) * D, kb * P:(kb + 1) * P],
                            rhs=QTP[p4][hh * D:(hh + 1) * D, qb * 512:(qb + 1) * 512],
                            start=True, stop=True)
                    # exp: head0 on ACT (exact), head1 on DVE (schraudolph)
                    pA0 = ppool.tile([P, 512], BF16, tag="P0")
                    nc.scalar.activation(out=pA0[:], in_=s2t[:, 0, :], func=AF.Exp,
                                         scale=rscl[:, kb:kb + 1])
                    pA1i = ppool.tile([P, 512], I16, tag="P1", bufs=5)
                    nc.vector.tensor_scalar(out=pA1i[:], in0=s2t[:, 1, :],
                                            scalar1=rsca[:, kb:kb + 1],
                                            scalar2=SCHB, op0=ALU.mult, op1=ALU.add)
                    pend0.append((kb, pA0))
                    pend1.append((kb, pA1i[:].bitcast(BF16)))
                    if len(pend0) > 2:
                        pkb, pp0 = pend0.pop(0)
                        nc.tensor.matmul(oo[0][:], lhsT=VP[:, pkb, 2 * p4, :], rhs=pp0[:],
                                         start=(pkb == 0), stop=(pkb == 31))
                    if len(pend1) > 3:
                        attn_v1(*pend1.pop(0))
                for pkb, pp0 in pend0:
                    nc.tensor.matmul(oo[0][:], lhsT=VP[:, pkb, 2 * p4, :], rhs=pp0[:],
                                     start=(pkb == 0), stop=(pkb == 31))
                for pkb, pp1 in pend1:
                    attn_v1(pkb, pp1)
                for hh in range(2):
                    dg = npool.tile([2, 512], F32, tag="den")
                    nc.scalar.copy(out=dg[:], in_=oo[hh][D:D + 2, :])
                    g0 = npool.tile([1, 512], F32, tag="g0")
                    nc.sync.dma_start(out=g0, in_=dg[1:2, :])
                    rec = npool.tile([1, 512], F32, tag="rec")
                    nc.vector.reciprocal_approx_fast(out=rec[:], in_=dg[0:1, :])
                    gsc = npool.tile([1, 512], F32, tag="gsc")
                    nc.vector.tensor_mul(out=gsc[:], in0=g0[:], in1=rec[:])
                    rc = npool.tile([P, 512], F32, tag="rc")
                    nc.gpsimd.partition_broadcast(out_ap=rc[:], in_ap=rec[:], channels=P)
                    gb = npool.tile([P, 512], F32, tag="gb")
                    nc.gpsimd.partition_broadcast(out_ap=gb[:], in_ap=gsc[:], channels=P)
                    hs = slice(hh * D, (hh + 1) * D)
                    ot = O1T[hs, p4, qb * 512:(qb + 1) * 512]
                    nc.vector.tensor_mul(out=ot, in0=oo[hh][0:D, :], in1=rc[hs, :])
                    nc.vector.scalar_tensor_tensor(
                        out=ot, in0=gb[hs, :], scalar=cvc[hs, 2 * p4 + hh:2 * p4 + hh + 1],
                        in1=ot, op0=ALU.mult, op1=ALU.add)

    # ---------------- Phase E: out-proj 1 + residual -> h1 (bf16) -------------
    cm_h1, pool_h1 = open_pool("pool_h1")       # residual stream HT (lives to end)
    H1T = pool_h1.tile([P, CC, Q], BF16, tag="H1T")
    with tc.tile_pool(name="prps", bufs=3, space="PSUM") as pp:
        for qb in range(2):
            for e in range(CC):
                ps = pp.tile([P, 512], F32, tag="ps")
                for c in range(CC):
                    nc.tensor.matmul(ps[:], lhsT=WO1[c][:, e * P:(e + 1) * P],
                                     rhs=O1T[:, c, qb * 512:(qb + 1) * 512],
                                     start=(c == 0), stop=(c == CC - 1))
                nc.vector.tensor_add(out=H1T[:, e, qb * 512:(qb + 1) * 512],
                                     in0=ps[:], in1=XRES[:, e, qb * 512:(qb + 1) * 512])
    cm_at.__exit__(None, None, None)

    # LN stats in transposed layout: returns r_b / s_b broadcast tiles [P, Q]
    def ln_stats_T(HT, rb_t, sb_t, lp, tagp):
        with tc.tile_pool(name=f"lnps{tagp}", bufs=1, space="PSUM") as pp:
            mrow = lp.tile([1, Q], F32, tag="mrow2", name=f"mrow_{tagp}")
            rrow = lp.tile([1, Q], F32, tag="rrow2", name=f"rrow_{tagp}")
            vrow = lp.tile([1, Q], F32, tag="vrow2", name=f"vrow_{tagp}")
            for qh in range(2):
                qs = slice(qh * 512, (qh + 1) * 512)
                ps1 = pp.tile([1, 512], F32, tag="lnps1", name=f"lnps1_{tagp}_{qh}")
                ps2 = pp.tile([1, 512], F32, tag="lnps2", name=f"lnps2_{tagp}_{qh}")
                for c in range(CC):
                    sq = lp.tile([P, 512], BF16, tag="sq2")
                    nc.vector.tensor_mul(out=sq[:], in0=HT[:, c, qs], in1=HT[:, c, qs])
                    nc.tensor.matmul(ps1[:], lhsT=ones_b[:], rhs=HT[:, c, qs],
                                     start=(c == 0), stop=(c == CC - 1))
                    nc.tensor.matmul(ps2[:], lhsT=ones_b[:], rhs=sq[:],
                                     start=(c == 0), stop=(c == CC - 1))
                nc.vector.tensor_scalar(out=mrow[:, qs], in0=ps1[:], scalar1=1.0 / DIM,
                                        scalar2=None, op0=ALU.mult)
                msq = lp.tile([1, 512], F32, tag="msq2")
                nc.vector.tensor_mul(out=msq[:], in0=mrow[:, qs], in1=mrow[:, qs])
                nc.vector.scalar_tensor_tensor(out=vrow[:, qs], in0=ps2[:], scalar=1.0 / DIM,
                                               in1=msq[:], op0=ALU.mult, op1=ALU.subtract)
            lnv = lp.tile([1, Q], F32, tag="lnv2", name=f"lnv_{tagp}")
            nc.scalar.activation(out=lnv[:], in_=vrow[:], func=AF.Ln, bias=eps_t[0:1, :])
            nc.scalar.activation(out=rrow[:], in_=lnv[:], func=AF.Exp, scale=-0.5)
            rrb = lp.tile([1, Q], BF16, tag="rrb2", name=f"rrb_{tagp}")
            srb = lp.tile([1, Q], BF16, tag="srb2", name=f"srb_{tagp}")
            nc.vector.tensor_copy(out=rrb[:], in_=rrow[:])
            nc.vector.tensor_mul(out=srb[:], in0=mrow[:], in1=rrow[:])
            nc.gpsimd.partition_broadcast(out_ap=rb_t[:], in_ap=rrb[:], channels=P)
            nc.gpsimd.partition_broadcast(out_ap=sb_t[:], in_ap=srb[:], channels=P)

    # ---------------- Phase F: cross-attention --------------------------------
    cm_mid, pool_mid = open_pool("pool_mid", side="right")
    r2_b = pool_mid.tile([P, Q], BF16, tag="r2_b")
    s2_b = pool_mid.tile([P, Q], BF16, tag="s2_b")
    with tc.tile_pool(name="ln2", bufs=2) as lp:
        ln_stats_T(H1T, r2_b, s2_b, lp, "ln2")

    O2T = pool_mid.tile([P, CC, Q], BF16, tag="O2T")
    Q2TP = [pool_mid.tile([P, Q], BF16, tag=f"Q2T{p}", name=f"Q2T{p}") for p in range(4)]
    with tc.tile_pool(name="c2ps", bufs=2, space="PSUM") as pp, \
         tc.tile_pool(name="c2sb", bufs=4) as sb:
        for qb in range(2):
            qs = slice(qb * 512, (qb + 1) * 512)
            for p4 in range(4):
                ps = pp.tile([P, 512], F32, tag="ps2")
                for c in range(CC):
                    nc.tensor.matmul(ps[:], lhsT=WQ2[c][:, p4 * P:(p4 + 1) * P],
                                     rhs=H1T[:, c, qs],
                                     start=(c == 0), stop=(c == CC - 1))
                # fold LN2 into the cast: q2 = r2*(h@W) then += s2*(-colsum)
                nc.vector.tensor_mul(out=Q2TP[p4][:, qs], in0=ps[:], in1=r2_b[:, qs])
                nc.vector.scalar_tensor_tensor(
                    out=Q2TP[p4][:, qs], in0=s2_b[:, qs], scalar=ncs_q2[:, p4:p4 + 1],
                    in1=Q2TP[p4][:, qs], op0=ALU.mult, op1=ALU.add)
            for h in range(H):
                p4, hh = h // 2, h % 2
                s2 = pp.tile([TCTX, 512], F32, tag="s2")
                nc.tensor.matmul(
                    s2[:],
                    lhsT=K2TP[p4][hh * D:(hh + 1) * D, :],
                    rhs=Q2TP[p4][hh * D:(hh + 1) * D, qs],
                    start=True, stop=True)
                p2 = sb.tile([TCTX, 512], BF16, tag="p2")
                nc.scalar.activation(out=p2[:], in_=s2[:], func=AF.Exp, scale=SCALE)
                o65 = pp.tile([D + 1, 512], F32, tag="o65x")
                nc.tensor.matmul(o65[:], lhsT=VP2[:, h, :], rhs=p2[:], start=True, stop=True)
                den = sb.tile([1, 512], F32, tag="den2")
                nc.scalar.copy(out=den[:], in_=o65[D:D + 1, :])
                dbc = sb.tile([D, 512], F32, tag="dbc2")
                nc.gpsimd.partition_broadcast(out_ap=dbc[:], in_ap=den[:], channels=D)
                rc = sb.tile([D, 512], F32, tag="rc2")
                nc.vector.reciprocal_approx_fast(out=rc[:], in_=dbc[:])
                nc.vector.tensor_mul(
                    out=O2T[hh * D:(hh + 1) * D, p4, qs],
                    in0=o65[0:D, :], in1=rc[:])

    with tc.tile_pool(name="pr2ps", bufs=3, space="PSUM") as pp:
        for qb in range(2):
            for e in range(CC):
                ps = pp.tile([P, 512], F32, tag="ps")
                for c in range(CC):
                    nc.tensor.matmul(ps[:], lhsT=WO2[c][:, e * P:(e + 1) * P],
                                     rhs=O2T[:, c, qb * 512:(qb + 1) * 512],
                                     start=(c == 0), stop=(c == CC - 1))
                nc.vector.tensor_add(out=H1T[:, e, qb * 512:(qb + 1) * 512],
                                     in0=ps[:], in1=H1T[:, e, qb * 512:(qb + 1) * 512])
    H2T = H1T  # h2 written in place

    cm_mid.__exit__(None, None, None)
    cm_ffn, pool_ffn = open_pool("pool_ffn", side="right")  # H2NT, FF : to the end
    r3_b = pool_ffn.tile([P, Q], BF16, tag="r3_b")
    s3_b = pool_ffn.tile([P, Q], BF16, tag="s3_b")
    with tc.tile_pool(name="ln3", bufs=2) as lp:
        ln_stats_T(H2T, r3_b, s3_b, lp, "ln3")
    H2NT = pool_ffn.tile([P, CC, Q], BF16, tag="H2NT")
    with tc.tile_pool(name="ln3n", bufs=4) as lp:
        for qh in range(2):
            qs = slice(qh * 512, (qh + 1) * 512)
            for e in range(CC):
                # h2n = h2*r3 - s3  (s3 = m*r broadcast)
                tmp = lp.tile([P, 512], BF16, tag="tmp3")
                nc.vector.tensor_mul(out=tmp[:], in0=H2T[:, e, qs], in1=r3_b[:, qs])
                nc.vector.tensor_sub(out=H2NT[:, e, qs], in0=tmp[:], in1=s3_b[:, qs])

    # ---------------- Phase G: GEGLU FFN + out proj + residual ----------------
    FB = 16  # 2048/128 blocks in each geglu half
    FF = pool_ffn.tile([P, FB, Q], BF16, tag="FF")
    with tc.tile_pool(name="ffps", bufs=3, space="PSUM") as pp, \
         tc.tile_pool(name="outps", bufs=2, space="PSUM") as opp, \
         tc.tile_pool(name="ffsb", bufs=4) as sb:
        for qb in range(2):
            qs = slice(qb * 512, (qb + 1) * 512)
            for fb in range(FB):
                psy = pp.tile([P, 512], F32, tag="psy")
                psg = pp.tile([P, 512], F32, tag="psg")
                for c in range(CC):
                    nc.tensor.matmul(psy[:], lhsT=GW[c][:, fb * P:(fb + 1) * P],
                                     rhs=H2NT[:, c, qs],
                                     start=(c == 0), stop=(c == CC - 1))
                for c in range(CC):
                    nc.tensor.matmul(psg[:], lhsT=GW[c][:, 4 * DIM + fb * P:4 * DIM + (fb + 1) * P],
                                     rhs=H2NT[:, c, qs],
                                     start=(c == 0), stop=(c == CC - 1))
                ga = sb.tile([P, 512], BF16, tag="ga")
                nc.scalar.activation(out=ga[:], in_=psg[:], func=AF.Gelu_apprx_tanh)
                nc.vector.tensor_mul(out=FF[:, fb, qs], in0=psy[:], in1=ga[:])
            for e in range(CC):
                ps = opp.tile([P, 512], F32, tag="pso")
                for f in range(FB):
                    nc.tensor.matmul(ps[:], lhsT=OW[f][:, e * P:(e + 1) * P],
                                     rhs=FF[:, f, qs],
                                     start=(f == 0), stop=(f == FB - 1))
                fin = sb.tile([P, 512], F32, tag="fin")
                nc.vector.tensor_add(out=fin[:], in0=ps[:], in1=H2T[:, e, qs])
                nc.sync.dma_start(out=a["outT"][e * P:(e + 1) * P, qs], in_=fin[:])

    cm_ffn.__exit__(None, None, None)
    cm_h1.__exit__(None, None, None)
    cm_wl.__exit__(None, None, None)


def build_program():
    nc = bacc.Bacc("TRN2", target_bir_lowering=False, debug=False)
    a = {}

    def din(name, shape, dt):
        a[name] = nc.dram_tensor(name, list(shape), dt, kind="ExternalInput").ap()

    din("xT", [DIM, S], BF16)
    din("xnat", [S, DIM], BF16)
    din("ctxT", [DIM, TCTX], BF16)
    for w in ["wq1", "wk1", "wv1", "wo1", "wq2", "wk2", "wv2", "wo2"]:
        din(w, [DIM, DIM], BF16)
    din("gw", [DIM, 8 * DIM], BF16)
    din("ow", [4 * DIM, DIM], BF16)
    din("ncs_q1", [P, CC], F32)
    din("cs_k1", [P, CC], F32)
    din("ncs_q2", [P, CC], F32)
    din("cv_col", [P, H], F32)
    a["outT"] = nc.dram_tensor("outT", [DIM, Q], F32, kind="ExternalOutput").ap()

    with tile.TileContext(nc) as tc:
        with ExitStack() as ctx:
            _body(ctx, tc, a)
    nc.compile()
    return nc


def host_prepare(inputs):
    """Fold LN affine params into weights, cast, slice/permute per core."""
    f = lambda t: np.asarray(t, dtype=np.float32)
    x = f(inputs["x"])
    context = f(inputs["context"])
    g1 = f(inputs["ln1_g"])[:, None]
    g2 = f(inputs["ln2_g"])[:, None]
    g3 = f(inputs["ln3_g"])[:, None]
    for nm in ["ln1_b", "ln2_b", "ln3_b", "bo1", "bo2", "geglu_b", "out_b"]:
        assert not np.any(f(inputs[nm])), f"nonzero bias {nm} not supported"

    bf = ml_dtypes.bfloat16
    wq1 = g1 * f(inputs["wq1"])
    wk1 = g1 * f(inputs["wk1"])
    wv1 = g1 * f(inputs["wv1"])
    wq2 = g2 * f(inputs["wq2"])
    weights = {
        "wq1": wq1.astype(bf),
        "wk1": wk1.astype(bf),
        "wv1": wv1.astype(bf),
        "wo1": f(inputs["wo1"]).astype(bf),
        "wq2": wq2.astype(bf),
        "wk2": f(inputs["wk2"]).astype(bf),
        "wv2": f(inputs["wv2"]).astype(bf),
        "wo2": f(inputs["wo2"]).astype(bf),
        "gw": (g3 * f(inputs["geglu_w"])).astype(bf),
        "ow": f(inputs["out_w"]).astype(bf),
        # negated column sums for the LN fold-in, [128, 4] chunk-column layout
        "ncs_q1": np.ascontiguousarray(-wq1.sum(0).reshape(CC, P).T),
        "cs_k1": np.ascontiguousarray(wk1.sum(0).reshape(CC, P).T),
        "ncs_q2": np.ascontiguousarray(-wq2.sum(0).reshape(CC, P).T),
        "cv_col": np.ascontiguousarray(
            np.tile(wv1.sum(0).reshape(H, D).T, (2, 1))),
    }

    in_maps = []
    for core in range(NCORES):
        b = core // 4
        q0 = (core % 4) * Q
        perm = np.concatenate([np.arange(q0, q0 + Q), np.delete(np.arange(S), np.s_[q0:q0 + Q])])
        xc = x[b][perm]                       # [S, DIM], own queries first
        m = dict(weights)
        m["xT"] = np.ascontiguousarray(xc.T).astype(bf)
        m["xnat"] = np.ascontiguousarray(xc).astype(bf)
        m["ctxT"] = np.ascontiguousarray(context[b].T).astype(bf)
        in_maps.append(m)
    return in_maps


def kernel(**inputs):
    if "nc" not in _CACHE:
        _CACHE["nc"] = build_program()
    nc = _CACHE["nc"]
    in_maps = host_prepare(inputs)
    res = run_bass_kernel_spmd(nc, in_maps, list(range(NCORES)))
    out = np.zeros((B, S, DIM), dtype=np.float32)
    for core in range(NCORES):
        b = core // 4
        q0 = (core % 4) * Q
        out[b, q0:q0 + Q, :] = res.results[core]["outT"].T
    return out
